# revision 15
# baseline (speedup 1.0000x reference)
"""Trainium2 Bass kernel for the CVAE model (nn_CVAE_61658550501650).

Contract: kernel(**inputs) takes the FULL unsharded inputs of
reference.setup_inputs() and returns the full outputs
(x_output [32,9216], y_output [32,1], z [32,96], log_q [32,96]).

Strategy (8 NeuronCores, single NEFF, no collectives):
  - The fc (9216x9216) weight is sharded over its output dim: core c
    computes x_output[:, c*1152:(c+1)*1152]. Everything before fc is
    replicated (an AllReduce-based sharding measured ~200us of collective
    latency on this runtime — more than replication costs).
  - Weights are pre-cast to bf16 and pre-transposed/tiled on the host so
    every DMA is a big contiguous [P, free] copy; matmuls run bf16 with
    fp32 PSUM accumulation. Weight streams ride the sync-engine DMA
    queue; small constants ride gpsimd so they don't delay the streams.
  - Matmuls keep batch (32) on the PSUM partition dim with the weight as
    the 512-wide moving operand (N-cycles rule: wide moving = efficient).
  - where(is0,...) selects are mask multiplies; the group-dependent fc
    bias (b0/b1 @ fc_w.T + fc_b) is precomputed on host and added via
    K=1 broadcast matmuls with bf16 mask rows.
  - The sum_k W_k*outer(g_k,g_k) stage is done as 96 PE matmuls
    out_i[j,b] = sum_(b,k) G[(b,k),j] * (SEL[(b,k),b]*WG[(b,k),i]),
    with the masked moving operand built by one broadcast DVE multiply.
  - eps is jax.random.normal(key(42)) on the *default* jax backend,
    exactly mirroring reference.py (values are backend-dependent).
"""

import os
import numpy as np
import ml_dtypes

B = 32
N2 = 9216
H2 = 1024
H1 = 512
LT = 96          # latent == n_nodes
L = 4            # decoder layers
NC = 8           # cores
OC = N2 // NC    # 1152 fc output cols per core
KT1 = N2 // 128  # 72 k-tiles for enc1
W1B = 4          # enc1 k-tiles per DMA
KT2 = H2 // 128  # 8
KH = H1 // 128   # 4
IT = 96          # i-blocks of the fc contraction
FCB = 4          # fc i-blocks per DMA
OB = 16          # outer-product i-blocks per psum tile
LOG2PI = float(np.log(2.0 * np.pi))
FC_SCALE = 64.0  # fp8 fc weights are pre-scaled by this; undone in the Exp

bf16_np = ml_dtypes.bfloat16

_PROGRAM_CACHE = {}


def _build_program():
    if "nc" in _PROGRAM_CACHE:
        return _PROGRAM_CACHE["nc"]

    import concourse.bacc as bacc
    import concourse.mybir as mybir
    import concourse.tile as tile

    f32 = mybir.dt.float32
    bf16 = mybir.dt.bfloat16
    AF = mybir.ActivationFunctionType
    OP = mybir.AluOpType

    nc = bacc.Bacc(None, target_bir_lowering=False, debug=False, num_devices=NC)

    def din(name, shape, dtype=f32):
        return nc.dram_tensor(name, shape, dtype, kind="ExternalInput")

    # ---- inputs (host-preprocessed layouts) ----
    xT_d = din("xT", [128, KT1, B], bf16)
    ew1_d = din("ew1", [KT1 // W1B, 128, W1B * H2], bf16)
    b1r_d = din("b1r", [1, H2], bf16)
    ew2_d = din("ew2", [128, KT2, H1], bf16)
    b2r_d = din("b2r", [1, H1], bf16)
    hw_d = din("hw", [128, KH, 4 * LT], bf16)
    hbr_d = din("hbr", [1, 4 * LT], bf16)
    eps_d = din("eps", [B, LT])
    id32_d = din("id32", [B, B], bf16)
    regr_d = din("regr", [B, LT])
    ycol_d = din("ycol", [B, 1])
    decw_d = din("decw", [LT, L, LT], bf16)
    decb_d = din("decb", [LT, L])
    gw_d = din("gw", [LT, 2, L, LT], bf16)
    gcb_d = din("gcb", [1, 2 * L * LT], bf16)
    ones_d = din("ones", [1, B], bf16)
    m0c_d = din("m0c", [B, 1])
    m1c_d = din("m1c", [B, 1])
    m0r_d = din("m0r", [1, B], bf16)
    m1r_d = din("m1r", [1, B], bf16)
    # selw[4b+k, b'] = (b'==b) * (c[b]==0 ? W0[k] : W1[k])  (host-computed)
    selw_d = din("selw", [128, B], bf16)
    fcw_d = din("fcw", [IT // FCB, LT, FCB, OC], mybir.dt.float8e4)
    B0_d = din("B0", [1, OC], bf16)
    B1_d = din("B1", [1, OC], bf16)

    # ---- outputs ----
    xo_d = nc.dram_tensor("xo", [B, OC], f32, kind="ExternalOutput")
    z_d = nc.dram_tensor("z", [B, LT], f32, kind="ExternalOutput")
    lq_d = nc.dram_tensor("lq", [B, LT], f32, kind="ExternalOutput")
    y_d = nc.dram_tensor("y", [B, 1], f32, kind="ExternalOutput")

    FCCHUNKS = [(0, 512), (512, 1024), (1024, OC)]

    with tile.TileContext(nc) as tc:
        with (
            tc.tile_pool(name="cpool", bufs=1) as cp,
            tc.tile_pool(name="wpool", bufs=3) as wp,
            tc.tile_pool(name="fcpool", bufs=3) as fp,
        ):
            # ---- weight streams on the sync queue, in consumption order ----
            xT = cp.tile([128, KT1, B], bf16)
            nc.sync.dma_start(xT[:, :8, :], xT_d[:, :8, :])
            nc.sync.dma_start(xT[:, 8:, :], xT_d[:, 8:, :])
            # (ew1 tiles DMA'd inside the enc1 loop; ew2/hw after them, then fcw)
            ew2 = cp.tile([128, KT2, H1], bf16)
            hw = cp.tile([128, KH, 4 * LT], bf16)

            # ---- small constants on the gpsimd queue ----
            def cload(name, dram, shape, dtype):
                t = cp.tile(shape, dtype, name=name)
                nc.gpsimd.dma_start(t[:], dram[:])
                return t

            b1r = cload("b1r", b1r_d, [1, H2], bf16)
            b2r = cload("b2r", b2r_d, [1, H1], bf16)
            hbr = cload("hbr", hbr_d, [1, 4 * LT], bf16)
            eps = cload("eps", eps_d, [B, LT], f32)
            id32 = cload("id32", id32_d, [B, B], bf16)
            regr = cload("regr", regr_d, [B, LT], f32)
            ycol = cload("ycol", ycol_d, [B, 1], f32)
            decw = cload("decw", decw_d, [LT, L, LT], bf16)
            decb = cload("decb", decb_d, [LT, L], f32)
            gw = cload("gw", gw_d, [LT, 2, L, LT], bf16)
            gcb = cload("gcb", gcb_d, [1, 2 * L * LT], bf16)
            ones = cload("ones", ones_d, [1, B], bf16)
            m0c = cload("m0c", m0c_d, [B, 1], f32)
            m1c = cload("m1c", m1c_d, [B, 1], f32)
            m0r = cload("m0r", m0r_d, [1, B], bf16)
            m1r = cload("m1r", m1r_d, [1, B], bf16)
            selw = cload("selw", selw_d, [128, B], bf16)
            B0 = cload("B0", B0_d, [1, OC], bf16)
            B1 = cload("B1", B1_d, [1, OC], bf16)

            h1T = cp.tile([128, KT2, B], bf16)
            h2T = cp.tile([128, KH, B], bf16)

            with tc.tile_pool(name="psE", bufs=1, space="PSUM") as psE:
                # ================= encoder layer 1 =================
                # h1[b, h] = relu(x @ w1.T + b1): batch on psum partitions,
                # weight is the 512-wide moving operand.
                ph1 = psE.tile([B, H2], f32, tag="wide", bufs=1)
                for q2 in range(KT1 // W1B):
                    w1t = wp.tile(
                        [128, W1B * H2], bf16, tag="w1", name=f"w1_{q2}", bufs=6
                    )
                    nc.sync.dma_start(w1t[:], ew1_d[q2])
                    for s in range(W1B):
                        q = W1B * q2 + s
                        for n in range(0, H2, 512):
                            nc.tensor.matmul(
                                ph1[:, n:n + 512],
                                xT[:, q, :],
                                w1t[:, s * H2 + n:s * H2 + n + 512],
                                start=(q == 0),
                                stop=False,
                            )
                for n in range(0, H2, 512):
                    nc.tensor.matmul(
                        ph1[:, n:n + 512], ones[:], b1r[:, n:n + 512],
                        start=False, stop=True,
                    )
                nc.sync.dma_start(ew2[:], ew2_d[:])
                nc.sync.dma_start(hw[:], hw_d[:])
                h1 = cp.tile([B, H2], bf16)
                nc.scalar.activation(h1[:], ph1[:], AF.Relu)
                for m in range(KT2):
                    pt = psE.tile([128, B], bf16, tag="tr", name=f"pt1_{m}", bufs=2)
                    nc.tensor.transpose(pt[:], h1[:, m * 128:(m + 1) * 128], id32[:])
                    nc.scalar.activation(h1T[:, m, :], pt[:], AF.Copy)

                # ================= encoder layer 2 =================
                ph2 = psE.tile([B, H1], f32, tag="wide", bufs=1)
                for q in range(KT2):
                    nc.tensor.matmul(
                        ph2[:], h1T[:, q, :], ew2[:, q, :],
                        start=(q == 0), stop=False,
                    )
                nc.tensor.matmul(ph2[:], ones[:], b2r[:], start=False, stop=True)
                h2 = cp.tile([B, H1], bf16)
                nc.scalar.activation(h2[:], ph2[:], AF.Relu)
                for m in range(KH):
                    pt = psE.tile([128, B], bf16, tag="tr", name=f"pt2_{m}", bufs=2)
                    nc.tensor.transpose(pt[:], h2[:, m * 128:(m + 1) * 128], id32[:])
                    nc.scalar.activation(h2T[:, m, :], pt[:], AF.Copy)

                # ================= mu/logvar heads =================
                # head order: mu0, lv0, mu1, lv1 -> hd [32, 384]
                phd = psE.tile([B, 4 * LT], f32, tag="wide", bufs=1)
                for q in range(KH):
                    nc.tensor.matmul(
                        phd[:], h2T[:, q, :], hw[:, q, :],
                        start=(q == 0), stop=False,
                    )
                nc.tensor.matmul(phd[:], ones[:], hbr[:], start=False, stop=True)
                hd = cp.tile([B, 4 * LT], f32)
                nc.scalar.activation(hd[:], phd[:], AF.Copy)

                # ---- select mu/logvar by group mask ([32,1] scalars) ----
                mu = cp.tile([B, LT], f32)
                lv = cp.tile([B, LT], f32)
                ta = cp.tile([B, LT], f32)
                tb = cp.tile([B, LT], f32)
                nc.vector.tensor_scalar(ta[:], hd[:, 0:LT], m0c[:, 0:1], None, op0=OP.mult)
                nc.vector.tensor_scalar(tb[:], hd[:, 2 * LT:3 * LT], m1c[:, 0:1], None, op0=OP.mult)
                nc.vector.tensor_tensor(mu[:], ta[:], tb[:], op=OP.add)
                tc2 = cp.tile([B, LT], f32)
                td = cp.tile([B, LT], f32)
                nc.vector.tensor_scalar(tc2[:], hd[:, LT:2 * LT], m0c[:, 0:1], None, op0=OP.mult)
                nc.vector.tensor_scalar(td[:], hd[:, 3 * LT:4 * LT], m1c[:, 0:1], None, op0=OP.mult)
                nc.vector.tensor_tensor(lv[:], tc2[:], td[:], op=OP.add)

                # ---- reparameterize ----
                sd = cp.tile([B, LT], f32)
                nc.scalar.activation(sd[:], lv[:], AF.Exp, scale=0.5)
                esd = cp.tile([B, LT], f32)
                nc.vector.tensor_tensor(esd[:], eps[:], sd[:], op=OP.mult)
                z = cp.tile([B, LT], f32)
                nc.vector.tensor_tensor(z[:], mu[:], esd[:], op=OP.add)
                nc.gpsimd.dma_start(z_d[:], z[:])
                zm = cp.tile([B, LT], f32)
                nc.vector.tensor_tensor(zm[:], z[:], mu[:], op=OP.subtract)
                rec = cp.tile([B, LT], f32)
                nc.vector.reciprocal(rec[:], sd[:])
                r = cp.tile([B, LT], f32)
                nc.vector.tensor_tensor(r[:], zm[:], rec[:], op=OP.mult)
                r2 = cp.tile([B, LT], f32)
                nc.vector.tensor_tensor(r2[:], r[:], r[:], op=OP.mult)
                lns = cp.tile([B, LT], f32)
                nc.scalar.activation(lns[:], sd[:], AF.Ln)
                lqa = cp.tile([B, LT], f32)
                nc.vector.tensor_scalar(lqa[:], r2[:], -0.5, None, op0=OP.mult)
                lqb = cp.tile([B, LT], f32)
                nc.vector.tensor_tensor(lqb[:], lqa[:], lns[:], op=OP.subtract)
                lq = cp.tile([B, LT], f32)
                nc.vector.tensor_scalar(lq[:], lqb[:], -0.5 * LOG2PI, None, op0=OP.add)
                nc.gpsimd.dma_start(lq_d[:], lq[:])

                # ---- y output: rowwise dot(z, reg_w[:96]) + ycol ----
                ym = cp.tile([B, LT], f32)
                nc.vector.tensor_tensor(ym[:], z[:], regr[:], op=OP.mult)
                yr = cp.tile([B, 1], f32)
                nc.vector.reduce_sum(yr[:], ym[:], axis=mybir.AxisListType.X)
                yo = cp.tile([B, 1], f32)
                nc.vector.tensor_tensor(yo[:], yr[:], ycol[:], op=OP.add)
                nc.gpsimd.dma_start(y_d[:], yo[:])

                # ---- z.T (bf16) for the decoder ----
                zb = cp.tile([B, LT], bf16)
                nc.scalar.activation(zb[:], z[:], AF.Copy)
                pzt = psE.tile([LT, B], bf16, tag="tr", bufs=2)
                nc.tensor.transpose(pzt[:], zb[:], id32[:])
                zTb = cp.tile([LT, B], bf16)
                nc.scalar.activation(zTb[:], pzt[:], AF.Copy)

                # ================= decoder =================
                dT = []
                for k in range(L):
                    pd = psE.tile([LT, B], f32, tag="dg", name=f"pd{k}", bufs=2)
                    nc.tensor.matmul(pd[:], decw[:, k, :], zTb[:])
                    dk = cp.tile([LT, B], bf16, name=f"dT{k}")
                    nc.scalar.activation(dk[:], pd[:], AF.Sigmoid, bias=decb[:, k:k + 1])
                    dT.append(dk)

                # g paths + group select (bf16 throughout)
                gsb = []
                for k in range(L):
                    gpk = []
                    for p in range(2):
                        pg = psE.tile([B, LT], f32, tag="dg", name=f"pg{p}_{k}", bufs=2)
                        nc.tensor.matmul(pg[:], dT[k][:], gw[:, p, k, :], start=True, stop=False)
                        idx = (p * L + k) * LT
                        nc.tensor.matmul(
                            pg[:], ones[:], gcb[:, idx:idx + LT], start=False, stop=True
                        )
                        gg = cp.tile([B, LT], bf16, name=f"g{p}_{k}")
                        nc.scalar.activation(gg[:], pg[:], AF.Sigmoid)
                        gpk.append(gg)
                    gs0 = cp.tile([B, LT], bf16, name=f"gs0_{k}")
                    gsel = cp.tile([B, LT], bf16, name=f"gsel_{k}")
                    nc.vector.tensor_scalar(gs0[:], gpk[0][:], m0c[:, 0:1], None, op0=OP.mult)
                    nc.vector.tensor_scalar(gsel[:], gpk[1][:], m1c[:, 0:1], None, op0=OP.mult)
                    nc.vector.tensor_tensor(gsel[:], gsel[:], gs0[:], op=OP.add)
                    gsb.append(gsel)

                # scatter into (b,k)-partition layout: G_all[4b+k, j]
                G_all = cp.tile([128, LT], bf16)
                Gv = G_all.rearrange("(b k) j -> k b j", k=L)
                for k in range(L):
                    nc.gpsimd.dma_start(Gv[k], gsb[k][:])

                # masked moving operand for all 96 outer matmuls at once:
                # om[(b,k), i, b'] = G[(b,k), i] * selw[(b,k), b']
                om = cp.tile([128, LT, B], bf16)
                wgv = G_all[:].rearrange("p (i u) -> p i u", u=1).broadcast_to([128, LT, B])
                sv = selw[:].rearrange("p (u b) -> p u b", u=1).broadcast_to([128, LT, B])
                nc.vector.tensor_tensor(om[:], wgv, sv, op=OP.mult)

            # ================= outer products + big fc =================
            tT_blks = [
                cp.tile([LT, B, OB], bf16, name=f"tT{blk}")
                for blk in range(LT // OB)
            ]
            with (
                tc.tile_pool(name="psC", bufs=2, space="PSUM") as psC,
                tc.tile_pool(name="psF", bufs=1, space="PSUM") as psF,
            ):
                for blk in range(LT // OB):
                    po = psC.tile([LT, OB * B], f32, tag="outer", name=f"po{blk}", bufs=2)
                    for s in range(OB):
                        i = OB * blk + s
                        nc.tensor.matmul(
                            po[:, s * B:(s + 1) * B], G_all[:], om[:, i, :]
                        )
                    # po[j, (s,b)] -> tT_blks[blk][j, b, s]
                    nc.scalar.activation(
                        tT_blks[blk][:],
                        po[:].rearrange("j (s b) -> j b s", b=B),
                        AF.Copy,
                    )

                pf = psF.tile([B, OC], f32, tag="fc", bufs=1)
                for blk in range(IT // FCB):
                    fw = fp.tile([LT, FCB, OC], mybir.dt.float8e4, tag="fcw", name=f"fw{blk}", bufs=16)
                    nc.sync.dma_start(fw[:], fcw_d[blk])
                    for s in range(FCB):
                        i = FCB * blk + s
                        for (n0, n1) in FCCHUNKS:
                            nc.tensor.matmul(
                                pf[:, n0:n1],
                                tT_blks[i // OB][:, :, i % OB],
                                fw[:, s, n0:n1],
                                start=(i == 0),
                                stop=False,
                            )
                for (n0, n1) in FCCHUNKS:
                    nc.tensor.matmul(pf[:, n0:n1], m0r[:], B0[:, n0:n1], start=False, stop=False)
                    nc.tensor.matmul(pf[:, n0:n1], m1r[:], B1[:, n0:n1], start=False, stop=True)
                xo = cp.tile([B, OC], f32)
                for (n0, n1) in FCCHUNKS:
                    nc.scalar.activation(xo[:, n0:n1], pf[:, n0:n1], AF.Exp, scale=1.0 / FC_SCALE)
                    nc.gpsimd.dma_start(xo_d[:, n0:n1], xo[:, n0:n1])

    nc.compile()
    _PROGRAM_CACHE["nc"] = nc
    return nc


def _prep_inputs(inputs):
    f32 = np.float32

    def np32(x):
        return np.ascontiguousarray(np.asarray(x, dtype=f32))

    def bf(x):
        return np.ascontiguousarray(np.asarray(x, dtype=f32).astype(bf16_np))

    x = np32(inputs["x_input"])
    c = np32(inputs["c_input"])[:, 0]
    m0 = (c == 0).astype(f32)
    m1 = 1.0 - m0

    import jax
    import jax.numpy as jnp
    # Mirrors reference.py exactly: default-backend jax RNG.
    eps = np.asarray(
        jax.random.normal(jax.random.key(42), (B, LT), jnp.float32)
    ).astype(f32)

    enc_w1 = np32(inputs["enc_w1"])
    enc_w2 = np32(inputs["enc_w2"])
    hw_cat = np.concatenate(
        [np32(inputs["mu0_w"]), np32(inputs["lv0_w"]),
         np32(inputs["mu1_w"]), np32(inputs["lv1_w"])], axis=0
    )  # [384, 512]
    hb_cat = np.concatenate(
        [np32(inputs["mu0_b"]), np32(inputs["lv0_b"]),
         np32(inputs["mu1_b"]), np32(inputs["lv1_b"])]
    )  # [384]
    dec_w = np32(inputs["dec_w"])
    dec_b = np32(inputs["dec_b"])
    gc_w = np.stack([np32(inputs["gc0_w"]), np32(inputs["gc1_w"])])
    gc_b = np.stack([np32(inputs["gc0_b"]), np32(inputs["gc1_b"])])
    fc_w = np32(inputs["fc_w"])
    fc_b = np32(inputs["fc_b"])
    b0 = np32(inputs["b0"])
    b1 = np32(inputs["b1"])
    reg_w = np32(inputs["reg_w"])
    reg_b = np32(inputs["reg_b"])
    W0 = np32(inputs["W0"])[:, 0]
    W1 = np32(inputs["W1"])[:, 0]

    # enc1 weight: [q2, 128, 4*1024]: tile q=4q2+s rows f=128q+p, cols h
    ew1 = enc_w1.T.reshape(KT1 // W1B, W1B, 128, H2).transpose(0, 2, 1, 3)
    ew1 = ew1.reshape(KT1 // W1B, 128, W1B * H2)

    base = {
        "xT": bf(x.T.reshape(KT1, 128, B).transpose(1, 0, 2)),
        "ew1": bf(ew1),
        "b1r": bf(np32(inputs["enc_b1"])[None, :]),
        "ew2": bf(enc_w2.T.reshape(KT2, 128, H1).transpose(1, 0, 2)),
        "b2r": bf(np32(inputs["enc_b2"])[None, :]),
        "hw": bf(hw_cat.T.reshape(KH, 128, 4 * LT).transpose(1, 0, 2)),
        "hbr": bf(hb_cat[None, :]),
        "eps": np.ascontiguousarray(eps),
        "id32": np.eye(B, dtype=bf16_np),
        "regr": np.ascontiguousarray(np.broadcast_to(reg_w[0, :LT][None, :], (B, LT))),
        "ycol": np.ascontiguousarray((c * reg_w[0, LT] + reg_b[0])[:, None]),
        "decw": bf(dec_w.transpose(2, 0, 1)),
        "decb": np.ascontiguousarray(dec_b.T),
        "gw": bf(gc_w.transpose(3, 0, 1, 2)),
        "gcb": bf(gc_b.reshape(1, 2 * L * LT)),
        "ones": np.ones((1, B), dtype=bf16_np),
        "m0c": np.ascontiguousarray(m0[:, None]),
        "m1c": np.ascontiguousarray(m1[:, None]),
        "m0r": bf(m0[None, :]),
        "m1r": bf(m1[None, :]),
        "selw": bf(np.repeat(np.eye(B, dtype=f32), L, axis=0)
                   * np.where(m0[:, None] > 0, W0[None, :], W1[None, :]).reshape(128, 1)),
    }

    in_maps = []
    for cidx in range(NC):
        sl = slice(cidx * OC, (cidx + 1) * OC)
        fcw_c = fc_w[sl, :]                     # [1152, 9216]
        fcwT = fcw_c.T                          # [9216, 1152], f = i*96+j
        m = dict(base)
        m["fcw"] = np.ascontiguousarray(
            (fcwT.reshape(IT // FCB, FCB, LT, OC).transpose(0, 2, 1, 3) * FC_SCALE)
            .astype(ml_dtypes.float8_e4m3)
        )
        m["B0"] = bf((fcw_c @ b0 + fc_b[sl])[None, :] * FC_SCALE)
        m["B1"] = bf((fcw_c @ b1 + fc_b[sl])[None, :] * FC_SCALE)
        in_maps.append(m)
    return in_maps


def _run(inputs, trace=False, trace_cores=None):
    from concourse.bass_utils import run_bass_kernel_spmd

    nc = _build_program()
    in_maps = _prep_inputs(inputs)
    kw = {}
    if trace:
        kw = dict(trace=True, trace_cores=trace_cores or list(range(NC)))
    res = run_bass_kernel_spmd(nc, in_maps, list(range(NC)), **kw)
    x_output = np.concatenate(
        [res.results[cdx]["xo"] for cdx in range(NC)], axis=1
    ).astype(np.float32)
    y_output = res.results[0]["y"].astype(np.float32)
    z = res.results[0]["z"].astype(np.float32)
    log_q = res.results[0]["lq"].astype(np.float32)
    return (x_output, y_output, z, log_q), res


def kernel(**inputs):
    outs, _ = _run(inputs, trace=bool(os.environ.get("BASS_KERNEL_TRACE")))
    return outs


# revision 18
# speedup vs baseline: 1.0251x; 1.0251x over previous
"""Trainium2 Bass kernel for the CVAE model (nn_CVAE_61658550501650).

Contract: kernel(**inputs) takes the FULL unsharded inputs of
reference.setup_inputs() and returns the full outputs
(x_output [32,9216], y_output [32,1], z [32,96], log_q [32,96]).

Strategy (8 NeuronCores, single NEFF, no collectives):
  - The fc (9216x9216) weight is sharded over its output dim: core c
    computes x_output[:, c*1152:(c+1)*1152]. Everything before fc is
    replicated (an AllReduce-based sharding measured ~200us of collective
    latency on this runtime — more than replication costs).
  - Weights are pre-cast to bf16 and pre-transposed/tiled on the host so
    every DMA is a big contiguous [P, free] copy; matmuls run bf16 with
    fp32 PSUM accumulation. Weight streams ride the sync-engine DMA
    queue; small constants ride gpsimd so they don't delay the streams.
  - Matmuls keep batch (32) on the PSUM partition dim with the weight as
    the 512-wide moving operand (N-cycles rule: wide moving = efficient).
  - where(is0,...) selects are mask multiplies; the group-dependent fc
    bias (b0/b1 @ fc_w.T + fc_b) is precomputed on host and added via
    K=1 broadcast matmuls with bf16 mask rows.
  - The sum_k W_k*outer(g_k,g_k) stage is done as 96 PE matmuls
    out_i[j,b] = sum_(b,k) G[(b,k),j] * (SEL[(b,k),b]*WG[(b,k),i]),
    with the masked moving operand built by one broadcast DVE multiply.
  - eps is jax.random.normal(key(42)) on the *default* jax backend,
    exactly mirroring reference.py (values are backend-dependent).
"""

import os
import numpy as np
import ml_dtypes

B = 32
N2 = 9216
H2 = 1024
H1 = 512
LT = 96          # latent == n_nodes
L = 4            # decoder layers
NC = 8           # cores
OC = N2 // NC    # 1152 fc output cols per core
KT1 = N2 // 128  # 72 k-tiles for enc1
W1B = 4          # enc1 k-tiles per DMA
KT2 = H2 // 128  # 8
KH = H1 // 128   # 4
IT = 96          # i-blocks of the fc contraction
FCB = 4          # fc i-blocks per DMA
OB = 16          # outer-product i-blocks per psum tile
LOG2PI = float(np.log(2.0 * np.pi))
FC_SCALE = 64.0  # fp8 fc weights are pre-scaled by this; undone in the Exp

bf16_np = ml_dtypes.bfloat16

_PROGRAM_CACHE = {}


def _build_program():
    if "nc" in _PROGRAM_CACHE:
        return _PROGRAM_CACHE["nc"]

    import concourse.bacc as bacc
    import concourse.mybir as mybir
    import concourse.tile as tile

    f32 = mybir.dt.float32
    bf16 = mybir.dt.bfloat16
    AF = mybir.ActivationFunctionType
    OP = mybir.AluOpType

    nc = bacc.Bacc(None, target_bir_lowering=False, debug=False, num_devices=NC)

    def din(name, shape, dtype=f32):
        return nc.dram_tensor(name, shape, dtype, kind="ExternalInput")

    # ---- inputs (host-preprocessed layouts) ----
    xT_d = din("xT", [128, KT1, B], bf16)
    ew1_d = din("ew1", [KT1 // W1B, 128, W1B * H2], bf16)
    b1r_d = din("b1r", [1, H2], bf16)
    ew2_d = din("ew2", [128, KT2, H1], bf16)
    b2r_d = din("b2r", [1, H1], bf16)
    hw_d = din("hw", [128, KH, 4 * LT], bf16)
    hbr_d = din("hbr", [1, 4 * LT], bf16)
    eps_d = din("eps", [B, LT])
    id32_d = din("id32", [B, B], bf16)
    regr_d = din("regr", [B, LT])
    ycol_d = din("ycol", [B, 1])
    decw_d = din("decw", [LT, L, LT], bf16)
    decb_d = din("decb", [LT, L])
    gw_d = din("gw", [LT, 2, L, LT], bf16)
    gcb_d = din("gcb", [1, 2 * L * LT], bf16)
    ones_d = din("ones", [1, B], bf16)
    m0c_d = din("m0c", [B, 1])
    m1c_d = din("m1c", [B, 1])
    m0r_d = din("m0r", [1, B], bf16)
    m1r_d = din("m1r", [1, B], bf16)
    # selw[k*32+b, b'] = (b'==b) * (c[b]==0 ? W0[k] : W1[k])  (host-computed)
    selw_d = din("selw", [128, B], bf16)
    m0k_d = din("m0k", [3 * B, 1])
    m1k_d = din("m1k", [3 * B, 1])
    fcw_d = din("fcw", [IT // FCB, LT, FCB, OC], mybir.dt.float8e4)
    B0_d = din("B0", [1, OC], bf16)
    B1_d = din("B1", [1, OC], bf16)

    # ---- outputs ----
    xo_d = nc.dram_tensor("xo", [B, OC], f32, kind="ExternalOutput")
    z_d = nc.dram_tensor("z", [B, LT], f32, kind="ExternalOutput")
    lq_d = nc.dram_tensor("lq", [B, LT], f32, kind="ExternalOutput")
    y_d = nc.dram_tensor("y", [B, 1], f32, kind="ExternalOutput")

    FCCHUNKS = [(0, 512), (512, 1024), (1024, OC)]

    with tile.TileContext(nc) as tc:
        with (
            tc.tile_pool(name="cpool", bufs=1) as cp,
            tc.tile_pool(name="wpool", bufs=3) as wp,
            tc.tile_pool(name="fcpool", bufs=3) as fp,
        ):
            # ---- weight streams on the sync queue, in consumption order ----
            xT = cp.tile([128, KT1, B], bf16)
            nc.sync.dma_start(xT[:, :8, :], xT_d[:, :8, :])
            nc.sync.dma_start(xT[:, 8:, :], xT_d[:, 8:, :])
            # (ew1 tiles DMA'd inside the enc1 loop; ew2/hw after them, then fcw)
            ew2 = cp.tile([128, KT2, H1], bf16)
            hw = cp.tile([128, KH, 4 * LT], bf16)

            # ---- small constants on the gpsimd queue ----
            def cload(name, dram, shape, dtype):
                t = cp.tile(shape, dtype, name=name)
                nc.gpsimd.dma_start(t[:], dram[:])
                return t

            b1r = cload("b1r", b1r_d, [1, H2], bf16)
            b2r = cload("b2r", b2r_d, [1, H1], bf16)
            hbr = cload("hbr", hbr_d, [1, 4 * LT], bf16)
            eps = cload("eps", eps_d, [B, LT], f32)
            id32 = cload("id32", id32_d, [B, B], bf16)
            regr = cload("regr", regr_d, [B, LT], f32)
            ycol = cload("ycol", ycol_d, [B, 1], f32)
            decw = cload("decw", decw_d, [LT, L, LT], bf16)
            decb = cload("decb", decb_d, [LT, L], f32)
            gw = cload("gw", gw_d, [LT, 2, L, LT], bf16)
            gcb = cload("gcb", gcb_d, [1, 2 * L * LT], bf16)
            ones = cload("ones", ones_d, [1, B], bf16)
            m0c = cload("m0c", m0c_d, [B, 1], f32)
            m1c = cload("m1c", m1c_d, [B, 1], f32)
            m0r = cload("m0r", m0r_d, [1, B], bf16)
            m1r = cload("m1r", m1r_d, [1, B], bf16)
            selw = cload("selw", selw_d, [128, B], bf16)
            m0k = cload("m0k", m0k_d, [3 * B, 1], f32)
            m1k = cload("m1k", m1k_d, [3 * B, 1], f32)
            B0 = cload("B0", B0_d, [1, OC], bf16)
            B1 = cload("B1", B1_d, [1, OC], bf16)

            h1T = cp.tile([128, KT2, B], bf16)
            h2T = cp.tile([128, KH, B], bf16)

            with tc.tile_pool(name="psE", bufs=1, space="PSUM") as psE:
                # ================= encoder layer 1 =================
                # h1[b, h] = relu(x @ w1.T + b1): batch on psum partitions,
                # weight is the 512-wide moving operand.
                ph1 = psE.tile([B, H2], f32, tag="wide", bufs=1)
                for q2 in range(KT1 // W1B):
                    w1t = wp.tile(
                        [128, W1B * H2], bf16, tag="w1", name=f"w1_{q2}", bufs=6
                    )
                    nc.sync.dma_start(w1t[:], ew1_d[q2])
                    for s in range(W1B):
                        q = W1B * q2 + s
                        for n in range(0, H2, 512):
                            nc.tensor.matmul(
                                ph1[:, n:n + 512],
                                xT[:, q, :],
                                w1t[:, s * H2 + n:s * H2 + n + 512],
                                start=(q == 0),
                                stop=False,
                            )
                for n in range(0, H2, 512):
                    nc.tensor.matmul(
                        ph1[:, n:n + 512], ones[:], b1r[:, n:n + 512],
                        start=False, stop=True,
                    )
                nc.sync.dma_start(ew2[:], ew2_d[:])
                nc.sync.dma_start(hw[:], hw_d[:])
                h1 = cp.tile([B, H2], bf16)
                nc.scalar.activation(h1[:], ph1[:], AF.Relu)
                for m in range(KT2):
                    pt = psE.tile([128, B], bf16, tag="tr", name=f"pt1_{m}", bufs=2)
                    nc.tensor.transpose(pt[:], h1[:, m * 128:(m + 1) * 128], id32[:])
                    nc.scalar.activation(h1T[:, m, :], pt[:], AF.Copy)

                # ================= encoder layer 2 =================
                ph2 = psE.tile([B, H1], f32, tag="wide", bufs=1)
                for q in range(KT2):
                    nc.tensor.matmul(
                        ph2[:], h1T[:, q, :], ew2[:, q, :],
                        start=(q == 0), stop=False,
                    )
                nc.tensor.matmul(ph2[:], ones[:], b2r[:], start=False, stop=True)
                h2 = cp.tile([B, H1], bf16)
                nc.scalar.activation(h2[:], ph2[:], AF.Relu)
                for m in range(KH):
                    pt = psE.tile([128, B], bf16, tag="tr", name=f"pt2_{m}", bufs=2)
                    nc.tensor.transpose(pt[:], h2[:, m * 128:(m + 1) * 128], id32[:])
                    nc.scalar.activation(h2T[:, m, :], pt[:], AF.Copy)

                # ================= mu/logvar heads =================
                # head order: mu0, lv0, mu1, lv1 -> hd [32, 384]
                phd = psE.tile([B, 4 * LT], f32, tag="wide", bufs=1)
                for q in range(KH):
                    nc.tensor.matmul(
                        phd[:], h2T[:, q, :], hw[:, q, :],
                        start=(q == 0), stop=False,
                    )
                nc.tensor.matmul(phd[:], ones[:], hbr[:], start=False, stop=True)
                hd = cp.tile([B, 4 * LT], f32)
                nc.scalar.activation(hd[:], phd[:], AF.Copy)

                # ---- select mu||lv (contiguous [32,192]) by group mask ----
                mulv = cp.tile([B, 2 * LT], f32)
                ta = cp.tile([B, 2 * LT], f32)
                nc.vector.tensor_scalar(ta[:], hd[:, 0:2 * LT], m0c[:, 0:1], None, op0=OP.mult)
                nc.vector.tensor_scalar(mulv[:], hd[:, 2 * LT:4 * LT], m1c[:, 0:1], None, op0=OP.mult)
                nc.vector.tensor_tensor(mulv[:], mulv[:], ta[:], op=OP.add)
                mu = mulv[:, 0:LT]
                lv = mulv[:, LT:2 * LT]

                # ---- reparameterize ----
                sd = cp.tile([B, LT], f32)
                nc.scalar.activation(sd[:], lv, AF.Exp, scale=0.5)
                esd = cp.tile([B, LT], f32)
                nc.vector.tensor_tensor(esd[:], eps[:], sd[:], op=OP.mult)
                z = cp.tile([B, LT], f32)
                nc.vector.tensor_tensor(z[:], mu, esd[:], op=OP.add)
                nc.gpsimd.dma_start(z_d[:], z[:])
                zm = cp.tile([B, LT], f32)
                nc.vector.tensor_tensor(zm[:], z[:], mu, op=OP.subtract)
                rec = cp.tile([B, LT], f32)
                nc.vector.reciprocal(rec[:], sd[:])
                r = cp.tile([B, LT], f32)
                nc.vector.tensor_tensor(r[:], zm[:], rec[:], op=OP.mult)
                r2 = cp.tile([B, LT], f32)
                nc.vector.tensor_tensor(r2[:], r[:], r[:], op=OP.mult)
                lns = cp.tile([B, LT], f32)
                nc.scalar.activation(lns[:], sd[:], AF.Ln)
                lqa = cp.tile([B, LT], f32)
                nc.vector.tensor_scalar(lqa[:], r2[:], -0.5, None, op0=OP.mult)
                lqb = cp.tile([B, LT], f32)
                nc.vector.tensor_tensor(lqb[:], lqa[:], lns[:], op=OP.subtract)
                lq = cp.tile([B, LT], f32)
                nc.vector.tensor_scalar(lq[:], lqb[:], -0.5 * LOG2PI, None, op0=OP.add)
                nc.gpsimd.dma_start(lq_d[:], lq[:])

                # ---- y output: rowwise dot(z, reg_w[:96]) + ycol ----
                ym = cp.tile([B, LT], f32)
                nc.vector.tensor_tensor(ym[:], z[:], regr[:], op=OP.mult)
                yr = cp.tile([B, 1], f32)
                nc.vector.reduce_sum(yr[:], ym[:], axis=mybir.AxisListType.X)
                yo = cp.tile([B, 1], f32)
                nc.vector.tensor_tensor(yo[:], yr[:], ycol[:], op=OP.add)
                nc.gpsimd.dma_start(y_d[:], yo[:])

                # ---- z.T (bf16) for the decoder ----
                zb = cp.tile([B, LT], bf16)
                nc.scalar.activation(zb[:], z[:], AF.Copy)
                pzt = psE.tile([LT, B], bf16, tag="tr", bufs=2)
                nc.tensor.transpose(pzt[:], zb[:], id32[:])
                zTb = cp.tile([LT, B], bf16)
                nc.scalar.activation(zTb[:], pzt[:], AF.Copy)

                # ================= decoder =================
                dT = []
                for k in range(L):
                    pd = psE.tile([LT, B], f32, tag="dg", name=f"pd{k}", bufs=4)
                    nc.tensor.matmul(pd[:], decw[:, k, :], zTb[:])
                    dk = cp.tile([LT, B], bf16, name=f"dT{k}")
                    nc.scalar.activation(dk[:], pd[:], AF.Sigmoid, bias=decb[:, k:k + 1])
                    dT.append(dk)

                # g paths: k=0..2 matmul straight into psum partition
                # groups {0,32,64}; k=3 (base 96 not allowed) via one DMA.
                PG0 = psE.tile([3 * B, LT], f32, tag="dg", bufs=4)
                PG1 = psE.tile([3 * B, LT], f32, tag="dg", bufs=4)
                for p, PG in ((0, PG0), (1, PG1)):
                    for k in range(3):
                        idx = (p * L + k) * LT
                        nc.tensor.matmul(
                            PG[32 * k:32 * k + B, :], dT[k][:], gw[:, p, k, :],
                            start=True, stop=False,
                        )
                        nc.tensor.matmul(
                            PG[32 * k:32 * k + B, :], ones[:], gcb[:, idx:idx + LT],
                            start=False, stop=True,
                        )
                G0r = cp.tile([3 * B, LT], bf16)
                G1r = cp.tile([3 * B, LT], bf16)
                nc.scalar.activation(G0r[:], PG0[:], AF.Sigmoid)
                nc.scalar.activation(G1r[:], PG1[:], AF.Sigmoid)
                G_all = cp.tile([128, LT], bf16)
                t96 = cp.tile([3 * B, LT], bf16)
                nc.vector.tensor_scalar(t96[:], G0r[:], m0k[:, 0:1], None, op0=OP.mult)
                nc.vector.tensor_scalar(G_all[0:3 * B, :], G1r[:], m1k[:, 0:1], None, op0=OP.mult)
                nc.vector.tensor_tensor(G_all[0:3 * B, :], G_all[0:3 * B, :], t96[:], op=OP.add)
                # k = 3
                g3 = []
                for p in range(2):
                    pg = psE.tile([B, LT], f32, tag="dg", name=f"pg3_{p}", bufs=4)
                    idx = (p * L + 3) * LT
                    nc.tensor.matmul(pg[:], dT[3][:], gw[:, p, 3, :], start=True, stop=False)
                    nc.tensor.matmul(pg[:], ones[:], gcb[:, idx:idx + LT], start=False, stop=True)
                    gg = cp.tile([B, LT], bf16, name=f"g3_{p}")
                    nc.scalar.activation(gg[:], pg[:], AF.Sigmoid)
                    g3.append(gg)
                t32 = cp.tile([B, LT], bf16)
                g3s = cp.tile([B, LT], bf16)
                nc.vector.tensor_scalar(t32[:], g3[0][:], m0c[:, 0:1], None, op0=OP.mult)
                nc.vector.tensor_scalar(g3s[:], g3[1][:], m1c[:, 0:1], None, op0=OP.mult)
                nc.vector.tensor_tensor(g3s[:], g3s[:], t32[:], op=OP.add)
                nc.gpsimd.dma_start(G_all[3 * B:128, :], g3s[:])

                # masked moving operand for all 96 outer matmuls at once:
                # om[(b,k), i, b'] = G[(b,k), i] * selw[(b,k), b']
                om = cp.tile([128, LT, B], bf16)
                wgv = G_all[:].rearrange("p (i u) -> p i u", u=1).broadcast_to([128, LT, B])
                sv = selw[:].rearrange("p (u b) -> p u b", u=1).broadcast_to([128, LT, B])
                nc.vector.tensor_tensor(om[:], wgv, sv, op=OP.mult)

            # ================= outer products + big fc =================
            tT_blks = [
                cp.tile([LT, B, OB], bf16, name=f"tT{blk}")
                for blk in range(LT // OB)
            ]
            with (
                tc.tile_pool(name="psC", bufs=2, space="PSUM") as psC,
                tc.tile_pool(name="psF", bufs=1, space="PSUM") as psF,
            ):
                for blk in range(LT // OB):
                    po = psC.tile([LT, OB * B], f32, tag="outer", name=f"po{blk}", bufs=2)
                    for s in range(OB):
                        i = OB * blk + s
                        nc.tensor.matmul(
                            po[:, s * B:(s + 1) * B], G_all[:], om[:, i, :]
                        )
                    # po[j, (s,b)] -> tT_blks[blk][j, b, s]
                    nc.scalar.activation(
                        tT_blks[blk][:],
                        po[:].rearrange("j (s b) -> j b s", b=B),
                        AF.Copy,
                    )

                pf = psF.tile([B, OC], f32, tag="fc", bufs=1)
                for blk in range(IT // FCB):
                    fw = fp.tile([LT, FCB, OC], mybir.dt.float8e4, tag="fcw", name=f"fw{blk}", bufs=16)
                    nc.sync.dma_start(fw[:], fcw_d[blk])
                    for s in range(FCB):
                        i = FCB * blk + s
                        for (n0, n1) in FCCHUNKS:
                            nc.tensor.matmul(
                                pf[:, n0:n1],
                                tT_blks[i // OB][:, :, i % OB],
                                fw[:, s, n0:n1],
                                start=(i == 0),
                                stop=False,
                            )
                for (n0, n1) in FCCHUNKS:
                    nc.tensor.matmul(pf[:, n0:n1], m0r[:], B0[:, n0:n1], start=False, stop=False)
                    nc.tensor.matmul(pf[:, n0:n1], m1r[:], B1[:, n0:n1], start=False, stop=True)
                xo = cp.tile([B, OC], f32)
                for (n0, n1) in FCCHUNKS:
                    nc.scalar.activation(xo[:, n0:n1], pf[:, n0:n1], AF.Exp, scale=1.0 / FC_SCALE)
                    nc.gpsimd.dma_start(xo_d[:, n0:n1], xo[:, n0:n1])

    nc.compile()
    _PROGRAM_CACHE["nc"] = nc
    return nc


def _prep_inputs(inputs):
    f32 = np.float32

    def np32(x):
        return np.ascontiguousarray(np.asarray(x, dtype=f32))

    def bf(x):
        return np.ascontiguousarray(np.asarray(x, dtype=f32).astype(bf16_np))

    x = np32(inputs["x_input"])
    c = np32(inputs["c_input"])[:, 0]
    m0 = (c == 0).astype(f32)
    m1 = 1.0 - m0

    import jax
    import jax.numpy as jnp
    # Mirrors reference.py exactly: default-backend jax RNG.
    eps = np.asarray(
        jax.random.normal(jax.random.key(42), (B, LT), jnp.float32)
    ).astype(f32)

    enc_w1 = np32(inputs["enc_w1"])
    enc_w2 = np32(inputs["enc_w2"])
    hw_cat = np.concatenate(
        [np32(inputs["mu0_w"]), np32(inputs["lv0_w"]),
         np32(inputs["mu1_w"]), np32(inputs["lv1_w"])], axis=0
    )  # [384, 512]
    hb_cat = np.concatenate(
        [np32(inputs["mu0_b"]), np32(inputs["lv0_b"]),
         np32(inputs["mu1_b"]), np32(inputs["lv1_b"])]
    )  # [384]
    dec_w = np32(inputs["dec_w"])
    dec_b = np32(inputs["dec_b"])
    gc_w = np.stack([np32(inputs["gc0_w"]), np32(inputs["gc1_w"])])
    gc_b = np.stack([np32(inputs["gc0_b"]), np32(inputs["gc1_b"])])
    fc_w = np32(inputs["fc_w"])
    fc_b = np32(inputs["fc_b"])
    b0 = np32(inputs["b0"])
    b1 = np32(inputs["b1"])
    reg_w = np32(inputs["reg_w"])
    reg_b = np32(inputs["reg_b"])
    W0 = np32(inputs["W0"])[:, 0]
    W1 = np32(inputs["W1"])[:, 0]

    # enc1 weight: [q2, 128, 4*1024]: tile q=4q2+s rows f=128q+p, cols h
    ew1 = enc_w1.T.reshape(KT1 // W1B, W1B, 128, H2).transpose(0, 2, 1, 3)
    ew1 = ew1.reshape(KT1 // W1B, 128, W1B * H2)

    base = {
        "xT": bf(x.T.reshape(KT1, 128, B).transpose(1, 0, 2)),
        "ew1": bf(ew1),
        "b1r": bf(np32(inputs["enc_b1"])[None, :]),
        "ew2": bf(enc_w2.T.reshape(KT2, 128, H1).transpose(1, 0, 2)),
        "b2r": bf(np32(inputs["enc_b2"])[None, :]),
        "hw": bf(hw_cat.T.reshape(KH, 128, 4 * LT).transpose(1, 0, 2)),
        "hbr": bf(hb_cat[None, :]),
        "eps": np.ascontiguousarray(eps),
        "id32": np.eye(B, dtype=bf16_np),
        "regr": np.ascontiguousarray(np.broadcast_to(reg_w[0, :LT][None, :], (B, LT))),
        "ycol": np.ascontiguousarray((c * reg_w[0, LT] + reg_b[0])[:, None]),
        "decw": bf(dec_w.transpose(2, 0, 1)),
        "decb": np.ascontiguousarray(dec_b.T),
        "gw": bf(gc_w.transpose(3, 0, 1, 2)),
        "gcb": bf(gc_b.reshape(1, 2 * L * LT)),
        "ones": np.ones((1, B), dtype=bf16_np),
        "m0c": np.ascontiguousarray(m0[:, None]),
        "m1c": np.ascontiguousarray(m1[:, None]),
        "m0r": bf(m0[None, :]),
        "m1r": bf(m1[None, :]),
        "selw": bf(np.tile(np.eye(B, dtype=f32), (L, 1))
                   * np.where(m0[:, None] > 0, W0[None, :], W1[None, :]).T.reshape(128, 1)),
        "m0k": np.ascontiguousarray(np.tile(m0, 3)[:, None]),
        "m1k": np.ascontiguousarray(np.tile(m1, 3)[:, None]),
    }

    in_maps = []
    for cidx in range(NC):
        sl = slice(cidx * OC, (cidx + 1) * OC)
        fcw_c = fc_w[sl, :]                     # [1152, 9216]
        fcwT = fcw_c.T                          # [9216, 1152], f = i*96+j
        m = dict(base)
        m["fcw"] = np.ascontiguousarray(
            (fcwT.reshape(IT // FCB, FCB, LT, OC).transpose(0, 2, 1, 3) * FC_SCALE)
            .astype(ml_dtypes.float8_e4m3)
        )
        m["B0"] = bf((fcw_c @ b0 + fc_b[sl])[None, :] * FC_SCALE)
        m["B1"] = bf((fcw_c @ b1 + fc_b[sl])[None, :] * FC_SCALE)
        in_maps.append(m)
    return in_maps


def _run(inputs, trace=False, trace_cores=None):
    from concourse.bass_utils import run_bass_kernel_spmd

    nc = _build_program()
    in_maps = _prep_inputs(inputs)
    kw = {}
    if trace:
        kw = dict(trace=True, trace_cores=trace_cores or list(range(NC)))
    res = run_bass_kernel_spmd(nc, in_maps, list(range(NC)), **kw)
    x_output = np.concatenate(
        [res.results[cdx]["xo"] for cdx in range(NC)], axis=1
    ).astype(np.float32)
    y_output = res.results[0]["y"].astype(np.float32)
    z = res.results[0]["z"].astype(np.float32)
    log_q = res.results[0]["lq"].astype(np.float32)
    return (x_output, y_output, z, log_q), res


def kernel(**inputs):
    outs, _ = _run(inputs, trace=bool(os.environ.get("BASS_KERNEL_TRACE")))
    return outs
